# revision 7
# baseline (speedup 1.0000x reference)
"""FFJORD RK4 + Hutchinson trace kernel for 8x Trainium2 NeuronCores.

Strategy
--------
Pure data-parallel over the batch (65536 rows -> 8192 rows/core). Inside each
core, rows are processed in 8 "supertiles" of 1024 rows: two 512-row subtiles
(A, B) stacked on the 128 SBUF partitions (A feats on partitions 0-63, B on
64-127), features-major so the 3-layer MLP maps onto TensorE matmuls with the
batch as the moving (N=512) dimension.

The reference's finite-difference JVP is replaced by the analytic JVP
(identical for a piecewise-linear ReLU MLP up to rare kink crossings and the
reference's own fp32 cancellation noise ~1e-3):

    trace = e . (f(x + 0.5*eps_fd*e) - f(x)) / eps_fd  ~=  0.5 * e . (I + J_mlp) e

All matmuls run in float32r (TRN2 full-rate fp32 mode: inputs rounded to 11
mantissa bits, accumulation exact fp32), elementwise math in fp32. The scalar
time feature is folded into a per-step L1 bias table (b0 + t*W0[64]).
"""
import sys

sys.path.insert(0, "/opt/trn_rl_repo")

import numpy as np

import concourse.bass as bass
import concourse.tile as tile
from concourse import bacc, mybir
from concourse.bass_utils import run_bass_kernel_spmd

F32 = mybir.dt.float32
F32R = mybir.dt.float32r
AF = mybir.ActivationFunctionType
OP = mybir.AluOpType

NUM_STEPS = 16
FD_EPS = 1e-4
DT = 1.0 / NUM_STEPS
HALF_H = 0.5 * FD_EPS  # FD perturbation scale (folded into trace const)
D = 64
H = 256
N_CORES = 8
B_FULL = 65536
B_CORE = B_FULL // N_CORES  # 8192
BT = 512                    # batch columns per subtile
SUP_ROWS = 2 * BT           # rows per supertile (A|B stacked)
N_SUP = B_CORE // SUP_ROWS  # 8


def _round_f32r(x):
    """Round-to-nearest-even fp32 -> fp32r (11 explicit mantissa bits)."""
    u = np.ascontiguousarray(x, dtype=np.float32).view(np.uint32)
    lsb = (u >> 12) & 1
    u = (u + 0x7FF + lsb) & 0xFFFFF000
    return u.view(np.float32)


def _build(n_sup=N_SUP, n_steps=NUM_STEPS, b2_nonzero=False):
    nc = bacc.Bacc("TRN2", target_bir_lowering=False, debug=False,
                   enable_asserts=True, num_devices=N_CORES)
    rows = n_sup * SUP_ROWS

    x_d = nc.dram_tensor("x", [rows, D], F32, kind="ExternalInput").ap()
    e_d = nc.dram_tensor("eps", [n_steps, rows, D], F32, kind="ExternalInput").ap()
    w0_d = nc.dram_tensor("w0d", [128, H], F32R, kind="ExternalInput").ap()
    bt_d = nc.dram_tensor("b0t", [H, 2 * n_steps + 1], F32, kind="ExternalInput").ap()
    w1_d = nc.dram_tensor("w1", [H, H], F32R, kind="ExternalInput").ap()
    w2a_d = nc.dram_tensor("w2a", [H, 128], F32R, kind="ExternalInput").ap()
    w2b_d = nc.dram_tensor("w2b", [H, 128], F32R, kind="ExternalInput").ap()
    on_d = nc.dram_tensor("onesld", [128, 2], F32R, kind="ExternalInput").ap()
    id_d = nc.dram_tensor("ident", [128, 128], F32, kind="ExternalInput").ap()
    b2_d = nc.dram_tensor("b2d", [128, 1], F32, kind="ExternalInput").ap()
    xo_d = nc.dram_tensor("xo", [rows, D], F32, kind="ExternalOutput").ap()
    ld_d = nc.dram_tensor("ld", [rows], F32, kind="ExternalOutput").ap()

    with tile.TileContext(nc) as tc:
        with tc.tile_pool(name="wp", bufs=1) as wp, \
             tc.tile_pool(name="bm", bufs=2) as bmp, \
             tc.tile_pool(name="esb", bufs=2) as esp, \
             tc.tile_pool(name="hp", bufs=2) as hp, \
             tc.tile_pool(name="xs", bufs=2) as xsp, \
             tc.tile_pool(name="wk", bufs=2) as wk, \
             tc.tile_pool(name="pp", bufs=2) as ppool, \
             tc.tile_pool(name="zp", bufs=1, space="PSUM") as zp, \
             tc.tile_pool(name="mp", bufs=2, space="PSUM") as mp, \
             tc.tile_pool(name="tp", bufs=1, space="PSUM") as tp, \
             tc.tile_pool(name="trp", bufs=1, space="PSUM") as trp:

            # ---- weights / constants (loaded once) ----
            w0 = wp.tile([128, H], F32R, tag="w0")
            nc.sync.dma_start(w0[:], w0_d[:])
            w1t = []
            for kg in range(2):
                row = []
                for mg in range(2):
                    t = wp.tile([128, 128], F32R, tag=f"w1_{kg}{mg}")
                    nc.sync.dma_start(
                        t[:], w1_d[kg * 128:(kg + 1) * 128, mg * 128:(mg + 1) * 128])
                    row.append(t)
                w1t.append(row)
            w2t = {}
            for half, wd in (("a", w2a_d), ("b", w2b_d)):
                for kg in range(2):
                    t = wp.tile([128, 128], F32R, tag=f"w2_{half}{kg}")
                    nc.sync.dma_start(t[:], wd[kg * 128:(kg + 1) * 128, :])
                    w2t[(half, kg)] = t
            b0t = []
            for mg in range(2):
                t = wp.tile([128, 2 * n_steps + 1], F32, tag=f"b0t_{mg}")
                nc.sync.dma_start(t[:], bt_d[mg * 128:(mg + 1) * 128, :])
                b0t.append(t)
            ones = wp.tile([128, 2], F32R, tag="ones")
            nc.sync.dma_start(ones[:], on_d[:])
            ident = wp.tile([128, 128], F32, tag="ident")
            nc.sync.dma_start(ident[:], id_d[:])
            b2t = wp.tile([128, 1], F32, tag="b2t")
            nc.sync.dma_start(b2t[:], b2_d[:])
            b2s = b2t[:, 0:1] if b2_nonzero else 0.0

            def transpose_in(dst_ps, src_bm):
                """4x full PE transposes: interleaved [A_j|B_j] 128-col groups."""
                for j in range(4):
                    nc.tensor.transpose(
                        dst_ps[:, 128 * j:128 * j + 128],
                        src_bm[:, 128 * j:128 * j + 128],
                        ident[:, :])

            def mlp(xin, j, act_l2_dve):
                """One MLP eval: xin [128,512] f32r-rounded sbuf -> m psum [128,512].

                Returns (m_psum, h1, h2) - h's for the tangent masks."""
                xr = xin[:].bitcast(F32R)
                h1 = []
                for mg in range(2):
                    z = zp.tile([128, 2 * BT], F32, tag=f"z{mg}")
                    nc.tensor.matmul(z[:, 0:BT],
                                     w0[0:64, mg * 128:(mg + 1) * 128].bitcast(F32R),
                                     xr[0:64, :], start=True, stop=True)
                    nc.tensor.matmul(z[:, BT:2 * BT],
                                     w0[64:128, mg * 128:(mg + 1) * 128].bitcast(F32R),
                                     xr[64:128, :], start=True, stop=True)
                    h = hp.tile([128, 2 * BT], F32, tag=f"h1_{mg}")
                    nc.scalar.activation(h[:].bitcast(F32R), z[:], AF.Relu,
                                         bias=b0t[mg][:, j:j + 1])
                    h1.append(h)
                h2 = []
                for mg in range(2):
                    z = zp.tile([128, 2 * BT], F32, tag=f"z{mg}")
                    for kg in range(2):
                        st = (kg == 0)
                        sp = (kg == 1)
                        nc.tensor.matmul(z[:, 0:BT], w1t[kg][mg][:],
                                         h1[kg][:, 0:BT].bitcast(F32R),
                                         start=st, stop=sp)
                        nc.tensor.matmul(z[:, BT:2 * BT], w1t[kg][mg][:],
                                         h1[kg][:, BT:2 * BT].bitcast(F32R),
                                         start=st, stop=sp)
                    h = hp.tile([128, 2 * BT], F32, tag=f"h2_{mg}")
                    if act_l2_dve and mg == 1:
                        nc.vector.tensor_scalar_max(h[:].bitcast(F32R), z[:], 0.0)
                    else:
                        nc.scalar.activation(h[:].bitcast(F32R), z[:], AF.Relu)
                    h2.append(h)
                m = mp.tile([128, BT], F32, tag="m")
                first = True
                for half in ("a", "b"):
                    lo, hi = (0, BT) if half == "a" else (BT, 2 * BT)
                    for kg in range(2):
                        nc.tensor.matmul(m[:], w2t[(half, kg)][:],
                                         h2[kg][:, lo:hi].bitcast(F32R),
                                         start=first, stop=(half == "b" and kg == 1))
                        first = False
                return m, h1, h2

            def tangent(e_sb, h1, h2):
                """J_mlp @ e with relu' masks from h1/h2 -> jv psum [128, BT]."""
                er = e_sb[:].bitcast(F32R)
                v1 = []
                for mg in range(2):
                    u = zp.tile([128, 2 * BT], F32, tag=f"z{mg}")
                    nc.tensor.matmul(u[:, 0:BT],
                                     w0[0:64, mg * 128:(mg + 1) * 128].bitcast(F32R),
                                     er[0:64, :], start=True, stop=True)
                    nc.tensor.matmul(u[:, BT:2 * BT],
                                     w0[64:128, mg * 128:(mg + 1) * 128].bitcast(F32R),
                                     er[64:128, :], start=True, stop=True)
                    v = hp.tile([128, 2 * BT], F32, tag=f"v1_{mg}")
                    nc.vector.scalar_tensor_tensor(
                        v[:].bitcast(F32R), h1[mg][:], 0.0, u[:], OP.is_gt, OP.mult)
                    v1.append(v)
                v2 = []
                for mg in range(2):
                    u = zp.tile([128, 2 * BT], F32, tag=f"z{mg}")
                    for kg in range(2):
                        st = (kg == 0)
                        sp = (kg == 1)
                        nc.tensor.matmul(u[:, 0:BT], w1t[kg][mg][:],
                                         v1[kg][:, 0:BT].bitcast(F32R),
                                         start=st, stop=sp)
                        nc.tensor.matmul(u[:, BT:2 * BT], w1t[kg][mg][:],
                                         v1[kg][:, BT:2 * BT].bitcast(F32R),
                                         start=st, stop=sp)
                    v = hp.tile([128, 2 * BT], F32, tag=f"v2_{mg}")
                    nc.vector.scalar_tensor_tensor(
                        v[:].bitcast(F32R), h2[mg][:], 0.0, u[:], OP.is_gt, OP.mult)
                    v2.append(v)
                jv = mp.tile([128, BT], F32, tag="m")
                first = True
                for half in ("a", "b"):
                    lo, hi = (0, BT) if half == "a" else (BT, 2 * BT)
                    for kg in range(2):
                        nc.tensor.matmul(jv[:], w2t[(half, kg)][:],
                                         v2[kg][:, lo:hi].bitcast(F32R),
                                         start=first, stop=(half == "b" and kg == 1))
                        first = False
                return jv

            for sup in range(n_sup):
                r0 = sup * SUP_ROWS
                x_view = x_d[r0:r0 + SUP_ROWS, :].rearrange(
                    "(q p) d -> p q d", q=8)
                xo_view = xo_d[r0:r0 + SUP_ROWS, :].rearrange(
                    "(q p) d -> p q d", q=8)

                # --- load + transpose x -> xc [128, 512] ---
                xbm = bmp.tile([128, 512], F32, tag="xbm")
                nc.sync.dma_start(
                    xbm[:].rearrange("p (q d) -> p q d", q=8), x_view)
                pst = tp.tile([128, 512], F32, tag="pst")
                transpose_in(pst, xbm)
                xc = xsp.tile([128, BT], F32, tag="xst")
                nc.scalar.activation(xc[:].bitcast(F32R), pst[:], AF.Copy)

                trace_ps = trp.tile([2, BT], F32, tag="trace")

                for s in range(n_steps):
                    j0 = 2 * s
                    # --- eps load + transpose + round ---
                    ebm = bmp.tile([128, 512], F32, tag="ebm")
                    nc.sync.dma_start(
                        ebm[:].rearrange("p (q d) -> p q d", q=8),
                        e_d[s, r0:r0 + SUP_ROWS, :].rearrange(
                            "(q p) d -> p q d", q=8))
                    pse = tp.tile([128, 512], F32, tag="pst")
                    transpose_in(pse, ebm)
                    e_sb = esp.tile([128, BT], F32, tag="esb")
                    nc.scalar.activation(e_sb[:].bitcast(F32R), pse[:], AF.Copy)

                    # --- base eval + k1 ---
                    m1, h1, h2 = mlp(xc, j0, act_l2_dve=False)
                    k1 = wk.tile([128, BT], F32, tag="k1")
                    nc.vector.scalar_tensor_tensor(
                        k1[:], m1[:], b2s, xc[:], OP.add, OP.add)

                    # --- tangent + trace accumulation ---
                    jv = tangent(e_sb, h1, h2)
                    st_ = wk.tile([128, BT], F32, tag="st")
                    nc.vector.tensor_tensor(st_[:], e_sb[:], jv[:], op=OP.add)
                    p = ppool.tile([128, BT], F32, tag="p")
                    nc.vector.tensor_tensor(
                        p[:].bitcast(F32R), e_sb[:], st_[:], op=OP.mult)
                    nc.tensor.matmul(trace_ps[:], ones[:], p[:].bitcast(F32R),
                                     start=(s == 0), stop=(s == n_steps - 1),
                                     skip_group_check=True)

                    # --- k2 ---
                    x2 = wk.tile([128, BT], F32, tag="x2")
                    nc.vector.scalar_tensor_tensor(
                        x2[:].bitcast(F32R), k1[:], DT / 2, xc[:], OP.mult, OP.add)
                    m2, _, _ = mlp(x2, j0 + 1, act_l2_dve=True)
                    k2 = wk.tile([128, BT], F32, tag="k2")
                    nc.vector.scalar_tensor_tensor(
                        k2[:], m2[:], b2s, x2[:], OP.add, OP.add)

                    # --- k3 ---
                    x3 = wk.tile([128, BT], F32, tag="x3")
                    nc.vector.scalar_tensor_tensor(
                        x3[:].bitcast(F32R), k2[:], DT / 2, xc[:], OP.mult, OP.add)
                    m3, _, _ = mlp(x3, j0 + 1, act_l2_dve=True)
                    k3 = wk.tile([128, BT], F32, tag="k3")
                    nc.vector.scalar_tensor_tensor(
                        k3[:], m3[:], b2s, x3[:], OP.add, OP.add)

                    # --- k4 ---
                    x4 = wk.tile([128, BT], F32, tag="x4")
                    nc.vector.scalar_tensor_tensor(
                        x4[:].bitcast(F32R), k3[:], DT, xc[:], OP.mult, OP.add)
                    m4, _, _ = mlp(x4, j0 + 2, act_l2_dve=True)
                    k4 = wk.tile([128, BT], F32, tag="k4")
                    nc.vector.scalar_tensor_tensor(
                        k4[:], m4[:], b2s, x4[:], OP.add, OP.add)

                    # --- xn = xc + dt/6 * (k1 + 2k2 + 2k3 + k4) ---
                    u = wk.tile([128, BT], F32, tag="u")
                    nc.vector.tensor_tensor(u[:], k1[:], k4[:], op=OP.add)
                    v = wk.tile([128, BT], F32, tag="v")
                    nc.vector.tensor_tensor(v[:], k2[:], k3[:], op=OP.add)
                    w = wk.tile([128, BT], F32, tag="w")
                    nc.vector.scalar_tensor_tensor(
                        w[:], v[:], 2.0, u[:], OP.mult, OP.add)
                    xn = xsp.tile([128, BT], F32, tag="xst")
                    nc.vector.scalar_tensor_tensor(
                        xn[:].bitcast(F32R), w[:], DT / 6, xc[:], OP.mult, OP.add)
                    xc = xn

                # --- outputs: transpose xc back + trace DMA ---
                pso = tp.tile([128, 512], F32, tag="pst")
                for j in range(4):
                    nc.tensor.transpose(
                        pso[:, 128 * j:128 * j + 128],
                        xc[:, 128 * j:128 * j + 128],
                        ident[:, :])
                xob = bmp.tile([128, 512], F32, tag="xbm")
                nc.scalar.activation(xob[:], pso[:], AF.Copy)
                nc.sync.dma_start(
                    xo_view, xob[:].rearrange("p (q d) -> p q d", q=8))
                ld_view = ld_d[r0:r0 + SUP_ROWS].rearrange(
                    "(g sub k) -> sub g k", g=4, sub=2)
                ldb = ppool.tile([2, BT], F32, tag="ldb")
                nc.scalar.activation(ldb[:], trace_ps[:], AF.Copy)
                nc.sync.dma_start(ld_view, ldb[:])

    nc.compile()
    return nc


_CACHE = {}


def _get_nc(b2_nonzero):
    key = b2_nonzero
    if key not in _CACHE:
        _CACHE[key] = _build(b2_nonzero=b2_nonzero)
    return _CACHE[key]


def kernel(x, eps, W0, b0, W1, b1, W2, b2):
    x = np.asarray(x, np.float32)
    eps = np.asarray(eps, np.float32)
    W0 = np.asarray(W0, np.float32)
    b0 = np.asarray(b0, np.float32)
    W1 = np.asarray(W1, np.float32)
    b1 = np.asarray(b1, np.float32)
    W2 = np.asarray(W2, np.float32)
    b2 = np.asarray(b2, np.float32)

    b2_nonzero = bool(np.any(b2))
    nc = _get_nc(b2_nonzero)

    # host-side weight prep
    w0d = _round_f32r(np.concatenate([W0[:D], W0[:D]], axis=0))      # [128, 256]
    # L1 bias table: col j = b0 + t_j * W0[64], t_j = j*dt/2, plus b1 folded? no.
    tgrid = (np.arange(2 * NUM_STEPS + 1, dtype=np.float64) * (DT / 2))
    b0t = (b0[None, :].astype(np.float64)
           + tgrid[:, None] * W0[D].astype(np.float64)).astype(np.float32)  # [33, H]
    b0t = np.ascontiguousarray(b0t.T)                                 # [H, 33]
    # NOTE: b1 folding - L2 bias is b1 (zeros in spec); if nonzero we add via
    # the relu bias path. Handled below by asserting zero for the fast path.
    if np.any(b1):
        raise NotImplementedError("nonzero b1 not supported by this kernel build")
    w1r = _round_f32r(W1)
    w2a = np.zeros((H, 128), np.float32)
    w2a[:, :D] = W2
    w2b = np.zeros((H, 128), np.float32)
    w2b[:, D:] = W2
    w2a = _round_f32r(w2a)
    w2b = _round_f32r(w2b)
    onesld = np.zeros((128, 2), np.float32)
    c_star = -DT * 0.5  # trace = 0.5*e.(e+Jmlp e); ld -= dt*trace
    onesld[0:64, 0] = c_star
    onesld[64:128, 1] = c_star
    onesld = _round_f32r(onesld)
    ident = np.eye(128, dtype=np.float32)
    b2d = np.concatenate([b2, b2]).reshape(128, 1).astype(np.float32)

    in_maps = []
    for c in range(N_CORES):
        r0 = c * B_CORE
        in_maps.append({
            "x": x[r0:r0 + B_CORE],
            "eps": np.ascontiguousarray(eps[:, r0:r0 + B_CORE, :]),
            "w0d": w0d, "b0t": b0t, "w1": w1r, "w2a": w2a, "w2b": w2b,
            "onesld": onesld, "ident": ident, "b2d": b2d,
        })
    res = run_bass_kernel_spmd(nc, in_maps, core_ids=list(range(N_CORES)))
    x_out = np.concatenate([res.results[c]["xo"] for c in range(N_CORES)], axis=0)
    log_det = np.concatenate([res.results[c]["ld"] for c in range(N_CORES)], axis=0)
    return x_out, log_det


# revision 9
# speedup vs baseline: 1.2818x; 1.2818x over previous
"""FFJORD RK4 + Hutchinson trace kernel for 8x Trainium2 NeuronCores.

Strategy
--------
Pure data-parallel over the batch (65536 rows -> 8192 rows/core). Inside each
core, rows are processed in 8 "supertiles" of 1024 rows: two 512-row subtiles
(A, B) stacked on the 128 SBUF partitions (A feats on partitions 0-63, B on
64-127), features-major so the 3-layer MLP maps onto TensorE matmuls with the
batch as the moving (N=512) dimension.

The reference's finite-difference JVP is replaced by the analytic JVP
(identical for a piecewise-linear ReLU MLP up to rare kink crossings and the
reference's own fp32 cancellation noise ~1e-3):

    trace = e . (f(x + 0.5*eps_fd*e) - f(x)) / eps_fd  ~=  0.5 * e . (I + J_mlp) e

All matmuls run in float32r (TRN2 full-rate fp32 mode: inputs rounded to 11
mantissa bits, accumulation exact fp32), elementwise math in fp32. The scalar
time feature is folded into a per-step L1 bias table (b0 + t*W0[64]).
"""
import sys

sys.path.insert(0, "/opt/trn_rl_repo")

import numpy as np

import concourse.bass as bass
import concourse.tile as tile
from concourse import bacc, mybir
from concourse.bass_utils import run_bass_kernel_spmd

F32 = mybir.dt.float32
F32R = mybir.dt.float32r
AF = mybir.ActivationFunctionType
OP = mybir.AluOpType

NUM_STEPS = 16
FD_EPS = 1e-4
DT = 1.0 / NUM_STEPS
HALF_H = 0.5 * FD_EPS  # FD perturbation scale (folded into trace const)
D = 64
H = 256
N_CORES = 8
B_FULL = 65536
B_CORE = B_FULL // N_CORES  # 8192
BT = 512                    # batch columns per subtile
SUP_ROWS = 2 * BT           # rows per supertile (A|B stacked)
N_SUP = B_CORE // SUP_ROWS  # 8


def _round_f32r(x):
    """Round-to-nearest-even fp32 -> fp32r (11 explicit mantissa bits)."""
    u = np.ascontiguousarray(x, dtype=np.float32).view(np.uint32)
    lsb = (u >> 12) & 1
    u = (u + 0x7FF + lsb) & 0xFFFFF000
    return u.view(np.float32)


def _build(n_sup=N_SUP, n_steps=NUM_STEPS, b2_nonzero=False):
    nc = bacc.Bacc("TRN2", target_bir_lowering=False, debug=False,
                   enable_asserts=True, num_devices=N_CORES)
    rows = n_sup * SUP_ROWS

    x_d = nc.dram_tensor("x", [rows, D], F32, kind="ExternalInput").ap()
    e_d = nc.dram_tensor("eps", [n_steps, rows, D], F32, kind="ExternalInput").ap()
    w0_d = nc.dram_tensor("w0d", [128, H], F32R, kind="ExternalInput").ap()
    bt_d = nc.dram_tensor("b0t", [H, 2 * n_steps + 1], F32, kind="ExternalInput").ap()
    w1_d = nc.dram_tensor("w1", [H, H], F32R, kind="ExternalInput").ap()
    w2a_d = nc.dram_tensor("w2a", [H, 128], F32R, kind="ExternalInput").ap()
    w2b_d = nc.dram_tensor("w2b", [H, 128], F32R, kind="ExternalInput").ap()
    on_d = nc.dram_tensor("onesld", [128, 2], F32R, kind="ExternalInput").ap()
    id_d = nc.dram_tensor("ident", [128, 128], F32, kind="ExternalInput").ap()
    b2_d = nc.dram_tensor("b2d", [128, 1], F32, kind="ExternalInput").ap()
    xo_d = nc.dram_tensor("xo", [rows, D], F32, kind="ExternalOutput").ap()
    ld_d = nc.dram_tensor("ld", [rows], F32, kind="ExternalOutput").ap()

    with tile.TileContext(nc) as tc:
        with tc.tile_pool(name="wp", bufs=1) as wp, \
             tc.tile_pool(name="bm", bufs=2) as bmp, \
             tc.tile_pool(name="esb", bufs=2) as esp, \
             tc.tile_pool(name="hp", bufs=2) as hp, \
             tc.tile_pool(name="xs", bufs=2) as xsp, \
             tc.tile_pool(name="wk", bufs=2) as wk, \
             tc.tile_pool(name="pp", bufs=2) as ppool, \
             tc.tile_pool(name="zp", bufs=1, space="PSUM") as zp, \
             tc.tile_pool(name="mp", bufs=2, space="PSUM") as mp, \
             tc.tile_pool(name="tp", bufs=1, space="PSUM") as tp, \
             tc.tile_pool(name="trp", bufs=1, space="PSUM") as trp:

            # ---- weights / constants (loaded once) ----
            w0 = wp.tile([128, H], F32R, tag="w0")
            nc.sync.dma_start(w0[:], w0_d[:])
            w1t = []
            for kg in range(2):
                row = []
                for mg in range(2):
                    t = wp.tile([128, 128], F32R, tag=f"w1_{kg}{mg}")
                    nc.sync.dma_start(
                        t[:], w1_d[kg * 128:(kg + 1) * 128, mg * 128:(mg + 1) * 128])
                    row.append(t)
                w1t.append(row)
            w2t = {}
            for half, wd in (("a", w2a_d), ("b", w2b_d)):
                for kg in range(2):
                    t = wp.tile([128, 128], F32R, tag=f"w2_{half}{kg}")
                    nc.sync.dma_start(t[:], wd[kg * 128:(kg + 1) * 128, :])
                    w2t[(half, kg)] = t
            b0t = []
            for mg in range(2):
                t = wp.tile([128, 2 * n_steps + 1], F32, tag=f"b0t_{mg}")
                nc.sync.dma_start(t[:], bt_d[mg * 128:(mg + 1) * 128, :])
                b0t.append(t)
            ones = wp.tile([128, 2], F32R, tag="ones")
            nc.sync.dma_start(ones[:], on_d[:])
            ident = wp.tile([128, 128], F32, tag="ident")
            nc.sync.dma_start(ident[:], id_d[:])
            b2t = wp.tile([128, 1], F32, tag="b2t")
            nc.sync.dma_start(b2t[:], b2_d[:])
            b2s = b2t[:, 0:1] if b2_nonzero else 0.0

            def transpose_in(dst_ps, src_bm):
                """4x full PE transposes: interleaved [A_j|B_j] 128-col groups."""
                for j in range(4):
                    nc.tensor.transpose(
                        dst_ps[:, 128 * j:128 * j + 128],
                        src_bm[:, 128 * j:128 * j + 128],
                        ident[:, :])

            def mlp(xin, j, act_l2_dve):
                """One MLP eval: xin [128,512] f32r-rounded sbuf -> m psum [128,512].

                Returns (m_psum, h1, h2) - h's for the tangent masks."""
                xr = xin[:].bitcast(F32R)
                h1 = []
                for mg in range(2):
                    z = zp.tile([128, 2 * BT], F32, tag=f"z{mg}")
                    nc.tensor.matmul(z[:, 0:BT],
                                     w0[0:64, mg * 128:(mg + 1) * 128].bitcast(F32R),
                                     xr[0:64, :], start=True, stop=True)
                    nc.tensor.matmul(z[:, BT:2 * BT],
                                     w0[64:128, mg * 128:(mg + 1) * 128].bitcast(F32R),
                                     xr[64:128, :], start=True, stop=True)
                    h = hp.tile([128, 2 * BT], F32, tag=f"h1_{mg}")
                    nc.scalar.activation(h[:].bitcast(F32R), z[:], AF.Relu,
                                         bias=b0t[mg][:, j:j + 1])
                    h1.append(h)
                h2 = []
                for mg in range(2):
                    z = zp.tile([128, 2 * BT], F32, tag=f"z{mg}")
                    for kg in range(2):
                        st = (kg == 0)
                        sp = (kg == 1)
                        nc.tensor.matmul(z[:, 0:BT], w1t[kg][mg][:],
                                         h1[kg][:, 0:BT].bitcast(F32R),
                                         start=st, stop=sp)
                        nc.tensor.matmul(z[:, BT:2 * BT], w1t[kg][mg][:],
                                         h1[kg][:, BT:2 * BT].bitcast(F32R),
                                         start=st, stop=sp)
                    h = hp.tile([128, 2 * BT], F32, tag=f"h2_{mg}")
                    if act_l2_dve and mg == 1:
                        nc.vector.tensor_scalar_max(h[:].bitcast(F32R), z[:], 0.0)
                    else:
                        nc.scalar.activation(h[:].bitcast(F32R), z[:], AF.Relu)
                    h2.append(h)
                m = mp.tile([128, BT], F32, tag="m")
                first = True
                for half in ("a", "b"):
                    lo, hi = (0, BT) if half == "a" else (BT, 2 * BT)
                    for kg in range(2):
                        nc.tensor.matmul(m[:], w2t[(half, kg)][:],
                                         h2[kg][:, lo:hi].bitcast(F32R),
                                         start=first, stop=(half == "b" and kg == 1))
                        first = False
                return m, h1, h2

            def tangent(e_sb, h1, h2):
                """J_mlp @ e with relu' masks from h1/h2 -> jv psum [128, BT]."""
                er = e_sb[:].bitcast(F32R)
                v1 = []
                for mg in range(2):
                    u = zp.tile([128, 2 * BT], F32, tag=f"z{mg}")
                    nc.tensor.matmul(u[:, 0:BT],
                                     w0[0:64, mg * 128:(mg + 1) * 128].bitcast(F32R),
                                     er[0:64, :], start=True, stop=True)
                    nc.tensor.matmul(u[:, BT:2 * BT],
                                     w0[64:128, mg * 128:(mg + 1) * 128].bitcast(F32R),
                                     er[64:128, :], start=True, stop=True)
                    v = hp.tile([128, 2 * BT], F32, tag=f"v1_{mg}")
                    nc.vector.scalar_tensor_tensor(
                        v[:].bitcast(F32R), h1[mg][:], 0.0, u[:], OP.is_gt, OP.mult)
                    v1.append(v)
                v2 = []
                for mg in range(2):
                    u = zp.tile([128, 2 * BT], F32, tag=f"z{mg}")
                    for kg in range(2):
                        st = (kg == 0)
                        sp = (kg == 1)
                        nc.tensor.matmul(u[:, 0:BT], w1t[kg][mg][:],
                                         v1[kg][:, 0:BT].bitcast(F32R),
                                         start=st, stop=sp)
                        nc.tensor.matmul(u[:, BT:2 * BT], w1t[kg][mg][:],
                                         v1[kg][:, BT:2 * BT].bitcast(F32R),
                                         start=st, stop=sp)
                    v = hp.tile([128, 2 * BT], F32, tag=f"v2_{mg}")
                    nc.vector.scalar_tensor_tensor(
                        v[:].bitcast(F32R), h2[mg][:], 0.0, u[:], OP.is_gt, OP.mult)
                    v2.append(v)
                jv = mp.tile([128, BT], F32, tag="m")
                first = True
                for half in ("a", "b"):
                    lo, hi = (0, BT) if half == "a" else (BT, 2 * BT)
                    for kg in range(2):
                        nc.tensor.matmul(jv[:], w2t[(half, kg)][:],
                                         v2[kg][:, lo:hi].bitcast(F32R),
                                         start=first, stop=(half == "b" and kg == 1))
                        first = False
                return jv

            for sup in range(n_sup):
                r0 = sup * SUP_ROWS
                x_view = x_d[r0:r0 + SUP_ROWS, :].rearrange(
                    "(q p) d -> p q d", q=8)
                xo_view = xo_d[r0:r0 + SUP_ROWS, :].rearrange(
                    "(q p) d -> p q d", q=8)

                # --- load + transpose x -> xc [128, 512] ---
                xbm = bmp.tile([128, 512], F32, tag="xbm")
                nc.sync.dma_start(
                    xbm[:].rearrange("p (q d) -> p q d", q=8), x_view)
                pst = tp.tile([128, 512], F32, tag="pst")
                transpose_in(pst, xbm)
                xc = xsp.tile([128, BT], F32, tag="xst")
                nc.scalar.activation(xc[:].bitcast(F32R), pst[:], AF.Copy)

                trace_ps = trp.tile([2, BT], F32, tag="trace")

                for s in range(n_steps):
                    j0 = 2 * s
                    # --- eps load + transpose + round ---
                    ebm = bmp.tile([128, 512], F32, tag="ebm")
                    nc.sync.dma_start(
                        ebm[:].rearrange("p (q d) -> p q d", q=8),
                        e_d[s, r0:r0 + SUP_ROWS, :].rearrange(
                            "(q p) d -> p q d", q=8))
                    pse = tp.tile([128, 512], F32, tag="pst")
                    transpose_in(pse, ebm)
                    e_sb = esp.tile([128, BT], F32, tag="esb")
                    nc.scalar.activation(e_sb[:].bitcast(F32R), pse[:], AF.Copy)

                    # --- base eval + k1 ---
                    m1, h1, h2 = mlp(xc, j0, act_l2_dve=False)
                    k1 = wk.tile([128, BT], F32, tag="k1")
                    nc.vector.scalar_tensor_tensor(
                        k1[:], m1[:], b2s, xc[:], OP.add, OP.add)

                    # --- tangent + trace accumulation ---
                    jv = tangent(e_sb, h1, h2)
                    st_ = wk.tile([128, BT], F32, tag="st")
                    nc.vector.tensor_tensor(st_[:], e_sb[:], jv[:], op=OP.add)
                    p = ppool.tile([128, BT], F32, tag="p")
                    nc.vector.tensor_tensor(
                        p[:].bitcast(F32R), e_sb[:], st_[:], op=OP.mult)
                    nc.tensor.matmul(trace_ps[:], ones[:], p[:].bitcast(F32R),
                                     start=(s == 0), stop=(s == n_steps - 1),
                                     skip_group_check=True)

                    # --- k2 ---
                    x2 = wk.tile([128, BT], F32, tag="x2")
                    nc.vector.scalar_tensor_tensor(
                        x2[:].bitcast(F32R), k1[:], DT / 2, xc[:], OP.mult, OP.add)
                    m2, _, _ = mlp(x2, j0 + 1, act_l2_dve=True)
                    k2 = wk.tile([128, BT], F32, tag="k2")
                    nc.vector.scalar_tensor_tensor(
                        k2[:], m2[:], b2s, x2[:], OP.add, OP.add)

                    # --- k3 ---
                    x3 = wk.tile([128, BT], F32, tag="x3")
                    nc.vector.scalar_tensor_tensor(
                        x3[:].bitcast(F32R), k2[:], DT / 2, xc[:], OP.mult, OP.add)
                    m3, _, _ = mlp(x3, j0 + 1, act_l2_dve=True)
                    k3 = wk.tile([128, BT], F32, tag="k3")
                    nc.vector.scalar_tensor_tensor(
                        k3[:], m3[:], b2s, x3[:], OP.add, OP.add)

                    # --- k4 ---
                    x4 = wk.tile([128, BT], F32, tag="x4")
                    nc.vector.scalar_tensor_tensor(
                        x4[:].bitcast(F32R), k3[:], DT, xc[:], OP.mult, OP.add)
                    m4, _, _ = mlp(x4, j0 + 2, act_l2_dve=True)
                    k4 = wk.tile([128, BT], F32, tag="k4")
                    nc.vector.scalar_tensor_tensor(
                        k4[:], m4[:], b2s, x4[:], OP.add, OP.add)

                    # --- xn = xc + dt/6 * (k1 + 2k2 + 2k3 + k4) ---
                    u = wk.tile([128, BT], F32, tag="u")
                    nc.vector.tensor_tensor(u[:], k1[:], k4[:], op=OP.add)
                    v = wk.tile([128, BT], F32, tag="v")
                    nc.vector.tensor_tensor(v[:], k2[:], k3[:], op=OP.add)
                    w = wk.tile([128, BT], F32, tag="w")
                    nc.vector.scalar_tensor_tensor(
                        w[:], v[:], 2.0, u[:], OP.mult, OP.add)
                    xn = xsp.tile([128, BT], F32, tag="xst")
                    nc.vector.scalar_tensor_tensor(
                        xn[:].bitcast(F32R), w[:], DT / 6, xc[:], OP.mult, OP.add)
                    xc = xn

                # --- outputs: transpose xc back + trace DMA ---
                pso = tp.tile([128, 512], F32, tag="pst")
                for j in range(4):
                    nc.tensor.transpose(
                        pso[:, 128 * j:128 * j + 128],
                        xc[:, 128 * j:128 * j + 128],
                        ident[:, :])
                xob = bmp.tile([128, 512], F32, tag="xbm")
                nc.scalar.activation(xob[:], pso[:], AF.Copy)
                nc.sync.dma_start(
                    xo_view, xob[:].rearrange("p (q d) -> p q d", q=8))
                ld_view = ld_d[r0:r0 + SUP_ROWS].rearrange(
                    "(g sub k) -> sub g k", g=4, sub=2)
                ldb = ppool.tile([2, BT], F32, tag="ldb")
                nc.scalar.activation(ldb[:], trace_ps[:], AF.Copy)
                nc.sync.dma_start(ld_view, ldb[:])

    nc.compile()
    return nc


_CACHE = {}


def _get_executor(b2_nonzero):
    """Build (once) and cache a jitted 8-core shard_map executor.

    Returns (run, in_names, out_names, out_shapes) where
    run(concat_inputs: list[np.ndarray]) -> list of per-output global arrays.
    """
    key = b2_nonzero
    if key in _CACHE:
        return _CACHE[key]
    import jax
    from jax.sharding import Mesh, PartitionSpec
    from jax.experimental.shard_map import shard_map
    from concourse import bass2jax
    from concourse.bass2jax import (_bass_exec_p, install_neuronx_cc_hook,
                                    partition_id_tensor)

    nc = _build(b2_nonzero=b2_nonzero)
    install_neuronx_cc_hook()

    part_name = nc.partition_id_tensor.name if nc.partition_id_tensor else None
    in_names, out_names, out_avals = [], [], []
    for alloc in nc.m.functions[0].allocations:
        if not isinstance(alloc, mybir.MemoryLocationSet):
            continue
        name = alloc.memorylocations[0].name
        if alloc.kind == "ExternalInput":
            if name != part_name:
                in_names.append(name)
        elif alloc.kind == "ExternalOutput":
            out_names.append(name)
            out_avals.append(jax.core.ShapedArray(
                tuple(alloc.tensor_shape), mybir.dt.np(alloc.dtype)))
    n_params = len(in_names)
    n_outs = len(out_names)
    all_in_names = in_names + out_names
    if part_name is not None:
        all_in_names = all_in_names + [part_name]

    def _body(*args):
        operands = list(args)
        if part_name is not None:
            operands.append(partition_id_tensor())
        outs = _bass_exec_p.bind(
            *operands,
            out_avals=tuple(out_avals),
            in_names=tuple(all_in_names),
            out_names=tuple(out_names),
            lowering_input_output_aliases=(),
            sim_require_finite=True,
            sim_require_nnan=True,
            nc=nc,
        )
        return tuple(outs)

    devices = jax.devices()[:N_CORES]
    mesh = Mesh(np.asarray(devices), ("core",))
    donate = tuple(range(n_params, n_params + n_outs))
    sharded = jax.jit(
        shard_map(_body, mesh=mesh,
                  in_specs=(PartitionSpec("core"),) * (n_params + n_outs),
                  out_specs=(PartitionSpec("core"),) * n_outs,
                  check_rep=False),
        donate_argnums=donate, keep_unused=True)

    def run(concat_inputs):
        zeros = [np.zeros((N_CORES * a.shape[0], *a.shape[1:]), a.dtype)
                 for a in out_avals]
        outs = sharded(*concat_inputs, *zeros)
        return [np.asarray(o) for o in outs]

    out_shapes = [tuple(a.shape) for a in out_avals]
    _CACHE[key] = (run, in_names, out_names, out_shapes)
    return _CACHE[key]


def kernel(x, eps, W0, b0, W1, b1, W2, b2):
    x = np.asarray(x, np.float32)
    eps = np.asarray(eps, np.float32)
    W0 = np.asarray(W0, np.float32)
    b0 = np.asarray(b0, np.float32)
    W1 = np.asarray(W1, np.float32)
    b1 = np.asarray(b1, np.float32)
    W2 = np.asarray(W2, np.float32)
    b2 = np.asarray(b2, np.float32)

    b2_nonzero = bool(np.any(b2))
    run, in_names, out_names, _ = _get_executor(b2_nonzero)

    # host-side weight prep
    w0d = _round_f32r(np.concatenate([W0[:D], W0[:D]], axis=0))      # [128, 256]
    # L1 bias table: col j = b0 + t_j * W0[64], t_j = j*dt/2, plus b1 folded? no.
    tgrid = (np.arange(2 * NUM_STEPS + 1, dtype=np.float64) * (DT / 2))
    b0t = (b0[None, :].astype(np.float64)
           + tgrid[:, None] * W0[D].astype(np.float64)).astype(np.float32)  # [33, H]
    b0t = np.ascontiguousarray(b0t.T)                                 # [H, 33]
    # NOTE: b1 folding - L2 bias is b1 (zeros in spec); if nonzero we add via
    # the relu bias path. Handled below by asserting zero for the fast path.
    if np.any(b1):
        raise NotImplementedError("nonzero b1 not supported by this kernel build")
    w1r = _round_f32r(W1)
    w2a = np.zeros((H, 128), np.float32)
    w2a[:, :D] = W2
    w2b = np.zeros((H, 128), np.float32)
    w2b[:, D:] = W2
    w2a = _round_f32r(w2a)
    w2b = _round_f32r(w2b)
    onesld = np.zeros((128, 2), np.float32)
    c_star = -DT * 0.5  # trace = 0.5*e.(e+Jmlp e); ld -= dt*trace
    onesld[0:64, 0] = c_star
    onesld[64:128, 1] = c_star
    onesld = _round_f32r(onesld)
    ident = np.eye(128, dtype=np.float32)
    b2d = np.concatenate([b2, b2]).reshape(128, 1).astype(np.float32)

    # Global (concatenated over cores) input arrays for shard_map:
    # per-core eps shard is eps[:, r0:r0+B_CORE, :]; concatenated along axis 0
    # that is eps transposed to [cores*steps, B_CORE, D].
    eps_g = np.ascontiguousarray(
        eps.reshape(NUM_STEPS, N_CORES, B_CORE, D).transpose(1, 0, 2, 3)
    ).reshape(N_CORES * NUM_STEPS, B_CORE, D)
    per_name = {
        "x": x,                               # [8*8192, 64] already global
        "eps": eps_g,
        "w0d": np.tile(w0d, (N_CORES, 1)),
        "b0t": np.tile(b0t, (N_CORES, 1)),
        "w1": np.tile(w1r, (N_CORES, 1)),
        "w2a": np.tile(w2a, (N_CORES, 1)),
        "w2b": np.tile(w2b, (N_CORES, 1)),
        "onesld": np.tile(onesld, (N_CORES, 1)),
        "ident": np.tile(ident, (N_CORES, 1)),
        "b2d": np.tile(b2d, (N_CORES, 1)),
    }
    outs = run([per_name[n] for n in in_names])
    res = dict(zip(out_names, outs))
    x_out = res["xo"]
    log_det = res["ld"]
    return x_out, log_det


# revision 10
# speedup vs baseline: 108.0443x; 84.2942x over previous
"""FFJORD RK4 + Hutchinson trace kernel for 8x Trainium2 NeuronCores.

Strategy
--------
Pure data-parallel over the batch (65536 rows -> 8192 rows/core). Inside each
core, rows are processed in 8 "supertiles" of 1024 rows: two 512-row subtiles
(A, B) stacked on the 128 SBUF partitions (A feats on partitions 0-63, B on
64-127), features-major so the 3-layer MLP maps onto TensorE matmuls with the
batch as the moving (N=512) dimension.

The reference's finite-difference JVP is replaced by the analytic JVP
(identical for a piecewise-linear ReLU MLP up to rare kink crossings and the
reference's own fp32 cancellation noise ~1e-3):

    trace = e . (f(x + 0.5*eps_fd*e) - f(x)) / eps_fd  ~=  0.5 * e . (I + J_mlp) e

All matmuls run in float32r (TRN2 full-rate fp32 mode: inputs rounded to 11
mantissa bits, accumulation exact fp32), elementwise math in fp32. The scalar
time feature is folded into a per-step L1 bias table (b0 + t*W0[64]).
"""
import sys

sys.path.insert(0, "/opt/trn_rl_repo")

import numpy as np

import concourse.bass as bass
import concourse.tile as tile
from concourse import bacc, mybir
from concourse.bass_utils import run_bass_kernel_spmd

F32 = mybir.dt.float32
F32R = mybir.dt.float32r
AF = mybir.ActivationFunctionType
OP = mybir.AluOpType

NUM_STEPS = 16
FD_EPS = 1e-4
DT = 1.0 / NUM_STEPS
HALF_H = 0.5 * FD_EPS  # FD perturbation scale (folded into trace const)
D = 64
H = 256
N_CORES = 8
B_FULL = 65536
B_CORE = B_FULL // N_CORES  # 8192
BT = 512                    # batch columns per subtile
SUP_ROWS = 2 * BT           # rows per supertile (A|B stacked)
N_SUP = B_CORE // SUP_ROWS  # 8


def _round_f32r(x):
    """Round-to-nearest-even fp32 -> fp32r (11 explicit mantissa bits)."""
    u = np.ascontiguousarray(x, dtype=np.float32).view(np.uint32)
    lsb = (u >> 12) & 1
    u = (u + 0x7FF + lsb) & 0xFFFFF000
    return u.view(np.float32)


def _build(n_sup=N_SUP, n_steps=NUM_STEPS, b2_nonzero=False):
    nc = bacc.Bacc("TRN2", target_bir_lowering=False, debug=False,
                   enable_asserts=True, num_devices=N_CORES)
    rows = n_sup * SUP_ROWS

    x_d = nc.dram_tensor("x", [rows, D], F32, kind="ExternalInput").ap()
    e_d = nc.dram_tensor("eps", [n_steps, rows, D], F32, kind="ExternalInput").ap()
    w0_d = nc.dram_tensor("w0d", [128, H], F32R, kind="ExternalInput").ap()
    bt_d = nc.dram_tensor("b0t", [H, 2 * n_steps + 1], F32, kind="ExternalInput").ap()
    w1_d = nc.dram_tensor("w1", [H, H], F32R, kind="ExternalInput").ap()
    w2a_d = nc.dram_tensor("w2a", [H, 128], F32R, kind="ExternalInput").ap()
    w2b_d = nc.dram_tensor("w2b", [H, 128], F32R, kind="ExternalInput").ap()
    on_d = nc.dram_tensor("onesld", [128, 2], F32R, kind="ExternalInput").ap()
    id_d = nc.dram_tensor("ident", [128, 128], F32, kind="ExternalInput").ap()
    b2_d = nc.dram_tensor("b2d", [128, 1], F32, kind="ExternalInput").ap()
    xo_d = nc.dram_tensor("xo", [rows, D], F32, kind="ExternalOutput").ap()
    ld_d = nc.dram_tensor("ld", [rows], F32, kind="ExternalOutput").ap()

    with tile.TileContext(nc) as tc:
        with tc.tile_pool(name="wp", bufs=1) as wp, \
             tc.tile_pool(name="bm", bufs=2) as bmp, \
             tc.tile_pool(name="esb", bufs=2) as esp, \
             tc.tile_pool(name="hp", bufs=2) as hp, \
             tc.tile_pool(name="xs", bufs=2) as xsp, \
             tc.tile_pool(name="wk", bufs=2) as wk, \
             tc.tile_pool(name="pp", bufs=2) as ppool, \
             tc.tile_pool(name="zp", bufs=1, space="PSUM") as zp, \
             tc.tile_pool(name="mp", bufs=2, space="PSUM") as mp, \
             tc.tile_pool(name="tp", bufs=1, space="PSUM") as tp, \
             tc.tile_pool(name="trp", bufs=1, space="PSUM") as trp:

            # ---- weights / constants (loaded once) ----
            w0 = wp.tile([128, H], F32R, tag="w0")
            nc.sync.dma_start(w0[:], w0_d[:])
            w1t = []
            for kg in range(2):
                row = []
                for mg in range(2):
                    t = wp.tile([128, 128], F32R, tag=f"w1_{kg}{mg}")
                    nc.sync.dma_start(
                        t[:], w1_d[kg * 128:(kg + 1) * 128, mg * 128:(mg + 1) * 128])
                    row.append(t)
                w1t.append(row)
            w2t = {}
            for half, wd in (("a", w2a_d), ("b", w2b_d)):
                for kg in range(2):
                    t = wp.tile([128, 128], F32R, tag=f"w2_{half}{kg}")
                    nc.sync.dma_start(t[:], wd[kg * 128:(kg + 1) * 128, :])
                    w2t[(half, kg)] = t
            b0t = []
            for mg in range(2):
                t = wp.tile([128, 2 * n_steps + 1], F32, tag=f"b0t_{mg}")
                nc.sync.dma_start(t[:], bt_d[mg * 128:(mg + 1) * 128, :])
                b0t.append(t)
            ones = wp.tile([128, 2], F32R, tag="ones")
            nc.sync.dma_start(ones[:], on_d[:])
            ident = wp.tile([128, 128], F32, tag="ident")
            nc.sync.dma_start(ident[:], id_d[:])
            b2t = wp.tile([128, 1], F32, tag="b2t")
            nc.sync.dma_start(b2t[:], b2_d[:])
            b2s = b2t[:, 0:1] if b2_nonzero else 0.0

            def transpose_in(dst_ps, src_bm):
                """4x full PE transposes: interleaved [A_j|B_j] 128-col groups."""
                for j in range(4):
                    nc.tensor.transpose(
                        dst_ps[:, 128 * j:128 * j + 128],
                        src_bm[:, 128 * j:128 * j + 128],
                        ident[:, :])

            def mlp(xin, j, act_l2_dve):
                """One MLP eval: xin [128,512] f32r-rounded sbuf -> m psum [128,512].

                Returns (m_psum, h1, h2) - h's for the tangent masks."""
                xr = xin[:].bitcast(F32R)
                h1 = []
                for mg in range(2):
                    z = zp.tile([128, 2 * BT], F32, tag=f"z{mg}")
                    nc.tensor.matmul(z[:, 0:BT],
                                     w0[0:64, mg * 128:(mg + 1) * 128].bitcast(F32R),
                                     xr[0:64, :], start=True, stop=True)
                    nc.tensor.matmul(z[:, BT:2 * BT],
                                     w0[64:128, mg * 128:(mg + 1) * 128].bitcast(F32R),
                                     xr[64:128, :], start=True, stop=True)
                    h = hp.tile([128, 2 * BT], F32, tag=f"h1_{mg}")
                    nc.scalar.activation(h[:].bitcast(F32R), z[:], AF.Relu,
                                         bias=b0t[mg][:, j:j + 1])
                    h1.append(h)
                h2 = []
                for mg in range(2):
                    z = zp.tile([128, 2 * BT], F32, tag=f"z{mg}")
                    for kg in range(2):
                        st = (kg == 0)
                        sp = (kg == 1)
                        nc.tensor.matmul(z[:, 0:BT], w1t[kg][mg][:],
                                         h1[kg][:, 0:BT].bitcast(F32R),
                                         start=st, stop=sp)
                        nc.tensor.matmul(z[:, BT:2 * BT], w1t[kg][mg][:],
                                         h1[kg][:, BT:2 * BT].bitcast(F32R),
                                         start=st, stop=sp)
                    h = hp.tile([128, 2 * BT], F32, tag=f"h2_{mg}")
                    if act_l2_dve and mg == 1:
                        nc.vector.tensor_scalar_max(h[:].bitcast(F32R), z[:], 0.0)
                    else:
                        nc.scalar.activation(h[:].bitcast(F32R), z[:], AF.Relu)
                    h2.append(h)
                m = mp.tile([128, BT], F32, tag="m")
                first = True
                for half in ("a", "b"):
                    lo, hi = (0, BT) if half == "a" else (BT, 2 * BT)
                    for kg in range(2):
                        nc.tensor.matmul(m[:], w2t[(half, kg)][:],
                                         h2[kg][:, lo:hi].bitcast(F32R),
                                         start=first, stop=(half == "b" and kg == 1))
                        first = False
                return m, h1, h2

            def tangent(e_sb, h1, h2):
                """J_mlp @ e with relu' masks from h1/h2 -> jv psum [128, BT]."""
                er = e_sb[:].bitcast(F32R)
                v1 = []
                for mg in range(2):
                    u = zp.tile([128, 2 * BT], F32, tag=f"z{mg}")
                    nc.tensor.matmul(u[:, 0:BT],
                                     w0[0:64, mg * 128:(mg + 1) * 128].bitcast(F32R),
                                     er[0:64, :], start=True, stop=True)
                    nc.tensor.matmul(u[:, BT:2 * BT],
                                     w0[64:128, mg * 128:(mg + 1) * 128].bitcast(F32R),
                                     er[64:128, :], start=True, stop=True)
                    v = hp.tile([128, 2 * BT], F32, tag=f"v1_{mg}")
                    nc.vector.scalar_tensor_tensor(
                        v[:].bitcast(F32R), h1[mg][:], 0.0, u[:], OP.is_gt, OP.mult)
                    v1.append(v)
                v2 = []
                for mg in range(2):
                    u = zp.tile([128, 2 * BT], F32, tag=f"z{mg}")
                    for kg in range(2):
                        st = (kg == 0)
                        sp = (kg == 1)
                        nc.tensor.matmul(u[:, 0:BT], w1t[kg][mg][:],
                                         v1[kg][:, 0:BT].bitcast(F32R),
                                         start=st, stop=sp)
                        nc.tensor.matmul(u[:, BT:2 * BT], w1t[kg][mg][:],
                                         v1[kg][:, BT:2 * BT].bitcast(F32R),
                                         start=st, stop=sp)
                    v = hp.tile([128, 2 * BT], F32, tag=f"v2_{mg}")
                    nc.vector.scalar_tensor_tensor(
                        v[:].bitcast(F32R), h2[mg][:], 0.0, u[:], OP.is_gt, OP.mult)
                    v2.append(v)
                jv = mp.tile([128, BT], F32, tag="m")
                first = True
                for half in ("a", "b"):
                    lo, hi = (0, BT) if half == "a" else (BT, 2 * BT)
                    for kg in range(2):
                        nc.tensor.matmul(jv[:], w2t[(half, kg)][:],
                                         v2[kg][:, lo:hi].bitcast(F32R),
                                         start=first, stop=(half == "b" and kg == 1))
                        first = False
                return jv

            for sup in range(n_sup):
                r0 = sup * SUP_ROWS
                x_view = x_d[r0:r0 + SUP_ROWS, :].rearrange(
                    "(q p) d -> p q d", q=8)
                xo_view = xo_d[r0:r0 + SUP_ROWS, :].rearrange(
                    "(q p) d -> p q d", q=8)

                # --- load + transpose x -> xc [128, 512] ---
                xbm = bmp.tile([128, 512], F32, tag="xbm")
                nc.sync.dma_start(
                    xbm[:].rearrange("p (q d) -> p q d", q=8), x_view)
                pst = tp.tile([128, 512], F32, tag="pst")
                transpose_in(pst, xbm)
                xc = xsp.tile([128, BT], F32, tag="xst")
                nc.scalar.activation(xc[:].bitcast(F32R), pst[:], AF.Copy)

                trace_ps = trp.tile([2, BT], F32, tag="trace")

                for s in range(n_steps):
                    j0 = 2 * s
                    # --- eps load + transpose + round ---
                    ebm = bmp.tile([128, 512], F32, tag="ebm")
                    nc.sync.dma_start(
                        ebm[:].rearrange("p (q d) -> p q d", q=8),
                        e_d[s, r0:r0 + SUP_ROWS, :].rearrange(
                            "(q p) d -> p q d", q=8))
                    pse = tp.tile([128, 512], F32, tag="pst")
                    transpose_in(pse, ebm)
                    e_sb = esp.tile([128, BT], F32, tag="esb")
                    nc.scalar.activation(e_sb[:].bitcast(F32R), pse[:], AF.Copy)

                    # --- base eval + k1 ---
                    m1, h1, h2 = mlp(xc, j0, act_l2_dve=False)
                    k1 = wk.tile([128, BT], F32, tag="k1")
                    nc.vector.scalar_tensor_tensor(
                        k1[:], m1[:], b2s, xc[:], OP.add, OP.add)

                    # --- tangent + trace accumulation ---
                    jv = tangent(e_sb, h1, h2)
                    st_ = wk.tile([128, BT], F32, tag="st")
                    nc.vector.tensor_tensor(st_[:], e_sb[:], jv[:], op=OP.add)
                    p = ppool.tile([128, BT], F32, tag="p")
                    nc.vector.tensor_tensor(
                        p[:].bitcast(F32R), e_sb[:], st_[:], op=OP.mult)
                    nc.tensor.matmul(trace_ps[:], ones[:], p[:].bitcast(F32R),
                                     start=(s == 0), stop=(s == n_steps - 1),
                                     skip_group_check=True)

                    # --- k2 ---
                    x2 = wk.tile([128, BT], F32, tag="x2")
                    nc.vector.scalar_tensor_tensor(
                        x2[:].bitcast(F32R), k1[:], DT / 2, xc[:], OP.mult, OP.add)
                    m2, _, _ = mlp(x2, j0 + 1, act_l2_dve=True)
                    k2 = wk.tile([128, BT], F32, tag="k2")
                    nc.vector.scalar_tensor_tensor(
                        k2[:], m2[:], b2s, x2[:], OP.add, OP.add)

                    # --- k3 ---
                    x3 = wk.tile([128, BT], F32, tag="x3")
                    nc.vector.scalar_tensor_tensor(
                        x3[:].bitcast(F32R), k2[:], DT / 2, xc[:], OP.mult, OP.add)
                    m3, _, _ = mlp(x3, j0 + 1, act_l2_dve=True)
                    k3 = wk.tile([128, BT], F32, tag="k3")
                    nc.vector.scalar_tensor_tensor(
                        k3[:], m3[:], b2s, x3[:], OP.add, OP.add)

                    # --- k4 ---
                    x4 = wk.tile([128, BT], F32, tag="x4")
                    nc.vector.scalar_tensor_tensor(
                        x4[:].bitcast(F32R), k3[:], DT, xc[:], OP.mult, OP.add)
                    m4, _, _ = mlp(x4, j0 + 2, act_l2_dve=True)
                    k4 = wk.tile([128, BT], F32, tag="k4")
                    nc.vector.scalar_tensor_tensor(
                        k4[:], m4[:], b2s, x4[:], OP.add, OP.add)

                    # --- xn = xc + dt/6 * (k1 + 2k2 + 2k3 + k4) ---
                    u = wk.tile([128, BT], F32, tag="u")
                    nc.vector.tensor_tensor(u[:], k1[:], k4[:], op=OP.add)
                    v = wk.tile([128, BT], F32, tag="v")
                    nc.vector.tensor_tensor(v[:], k2[:], k3[:], op=OP.add)
                    w = wk.tile([128, BT], F32, tag="w")
                    nc.vector.scalar_tensor_tensor(
                        w[:], v[:], 2.0, u[:], OP.mult, OP.add)
                    xn = xsp.tile([128, BT], F32, tag="xst")
                    nc.vector.scalar_tensor_tensor(
                        xn[:].bitcast(F32R), w[:], DT / 6, xc[:], OP.mult, OP.add)
                    xc = xn

                # --- outputs: transpose xc back + trace DMA ---
                pso = tp.tile([128, 512], F32, tag="pst")
                for j in range(4):
                    nc.tensor.transpose(
                        pso[:, 128 * j:128 * j + 128],
                        xc[:, 128 * j:128 * j + 128],
                        ident[:, :])
                xob = bmp.tile([128, 512], F32, tag="xbm")
                nc.scalar.activation(xob[:], pso[:], AF.Copy)
                nc.sync.dma_start(
                    xo_view, xob[:].rearrange("p (q d) -> p q d", q=8))
                ld_view = ld_d[r0:r0 + SUP_ROWS].rearrange(
                    "(g sub k) -> sub g k", g=4, sub=2)
                ldb = ppool.tile([2, BT], F32, tag="ldb")
                nc.scalar.activation(ldb[:], trace_ps[:], AF.Copy)
                nc.sync.dma_start(ld_view, ldb[:])

    nc.compile()
    return nc


_CACHE = {}


def _get_executor(b2_nonzero):
    """Build (once) and cache a jitted 8-core shard_map executor.

    Returns (run, in_names, out_names, out_shapes) where
    run(concat_inputs: list[np.ndarray]) -> list of per-output global arrays.
    """
    key = b2_nonzero
    if key in _CACHE:
        return _CACHE[key]
    import jax
    from jax.sharding import Mesh, PartitionSpec
    from jax.experimental.shard_map import shard_map
    from concourse import bass2jax
    from concourse.bass2jax import (_bass_exec_p, install_neuronx_cc_hook,
                                    partition_id_tensor)

    nc = _build(b2_nonzero=b2_nonzero)
    install_neuronx_cc_hook()

    part_name = nc.partition_id_tensor.name if nc.partition_id_tensor else None
    in_names, out_names, out_avals = [], [], []
    for alloc in nc.m.functions[0].allocations:
        if not isinstance(alloc, mybir.MemoryLocationSet):
            continue
        name = alloc.memorylocations[0].name
        if alloc.kind == "ExternalInput":
            if name != part_name:
                in_names.append(name)
        elif alloc.kind == "ExternalOutput":
            out_names.append(name)
            out_avals.append(jax.core.ShapedArray(
                tuple(alloc.tensor_shape), mybir.dt.np(alloc.dtype)))
    n_params = len(in_names)
    n_outs = len(out_names)
    all_in_names = in_names + out_names
    if part_name is not None:
        all_in_names = all_in_names + [part_name]

    def _body(*args):
        operands = list(args)
        if part_name is not None:
            operands.append(partition_id_tensor())
        outs = _bass_exec_p.bind(
            *operands,
            out_avals=tuple(out_avals),
            in_names=tuple(all_in_names),
            out_names=tuple(out_names),
            lowering_input_output_aliases=(),
            sim_require_finite=True,
            sim_require_nnan=True,
            nc=nc,
        )
        return tuple(outs)

    devices = jax.devices()[:N_CORES]
    mesh = Mesh(np.asarray(devices), ("core",))
    donate = tuple(range(n_params, n_params + n_outs))
    sharded = jax.jit(
        shard_map(_body, mesh=mesh,
                  in_specs=(PartitionSpec("core"),) * (n_params + n_outs),
                  out_specs=(PartitionSpec("core"),) * n_outs,
                  check_rep=False),
        donate_argnums=donate, keep_unused=True)

    def run(concat_inputs):
        zeros = [np.zeros((N_CORES * a.shape[0], *a.shape[1:]), a.dtype)
                 for a in out_avals]
        outs = sharded(*concat_inputs, *zeros)
        return [np.asarray(o) for o in outs]

    out_shapes = [tuple(a.shape) for a in out_avals]
    _CACHE[key] = (run, in_names, out_names, out_shapes)
    _CACHE[(key, "bench")] = (sharded, out_avals)
    return _CACHE[key]


def kernel(x, eps, W0, b0, W1, b1, W2, b2):
    x = np.asarray(x, np.float32)
    eps = np.asarray(eps, np.float32)
    W0 = np.asarray(W0, np.float32)
    b0 = np.asarray(b0, np.float32)
    W1 = np.asarray(W1, np.float32)
    b1 = np.asarray(b1, np.float32)
    W2 = np.asarray(W2, np.float32)
    b2 = np.asarray(b2, np.float32)

    b2_nonzero = bool(np.any(b2))
    run, in_names, out_names, _ = _get_executor(b2_nonzero)

    # host-side weight prep
    w0d = _round_f32r(np.concatenate([W0[:D], W0[:D]], axis=0))      # [128, 256]
    # L1 bias table: col j = b0 + t_j * W0[64], t_j = j*dt/2, plus b1 folded? no.
    tgrid = (np.arange(2 * NUM_STEPS + 1, dtype=np.float64) * (DT / 2))
    b0t = (b0[None, :].astype(np.float64)
           + tgrid[:, None] * W0[D].astype(np.float64)).astype(np.float32)  # [33, H]
    b0t = np.ascontiguousarray(b0t.T)                                 # [H, 33]
    # NOTE: b1 folding - L2 bias is b1 (zeros in spec); if nonzero we add via
    # the relu bias path. Handled below by asserting zero for the fast path.
    if np.any(b1):
        raise NotImplementedError("nonzero b1 not supported by this kernel build")
    w1r = _round_f32r(W1)
    w2a = np.zeros((H, 128), np.float32)
    w2a[:, :D] = W2
    w2b = np.zeros((H, 128), np.float32)
    w2b[:, D:] = W2
    w2a = _round_f32r(w2a)
    w2b = _round_f32r(w2b)
    onesld = np.zeros((128, 2), np.float32)
    c_star = -DT * 0.5  # trace = 0.5*e.(e+Jmlp e); ld -= dt*trace
    onesld[0:64, 0] = c_star
    onesld[64:128, 1] = c_star
    onesld = _round_f32r(onesld)
    ident = np.eye(128, dtype=np.float32)
    b2d = np.concatenate([b2, b2]).reshape(128, 1).astype(np.float32)

    # Global (concatenated over cores) input arrays for shard_map:
    # per-core eps shard is eps[:, r0:r0+B_CORE, :]; concatenated along axis 0
    # that is eps transposed to [cores*steps, B_CORE, D].
    eps_g = np.ascontiguousarray(
        eps.reshape(NUM_STEPS, N_CORES, B_CORE, D).transpose(1, 0, 2, 3)
    ).reshape(N_CORES * NUM_STEPS, B_CORE, D)
    per_name = {
        "x": x,                               # [8*8192, 64] already global
        "eps": eps_g,
        "w0d": np.tile(w0d, (N_CORES, 1)),
        "b0t": np.tile(b0t, (N_CORES, 1)),
        "w1": np.tile(w1r, (N_CORES, 1)),
        "w2a": np.tile(w2a, (N_CORES, 1)),
        "w2b": np.tile(w2b, (N_CORES, 1)),
        "onesld": np.tile(onesld, (N_CORES, 1)),
        "ident": np.tile(ident, (N_CORES, 1)),
        "b2d": np.tile(b2d, (N_CORES, 1)),
    }
    outs = run([per_name[n] for n in in_names])
    res = dict(zip(out_names, outs))
    x_out = res["xo"]
    log_det = res["ld"]
    return x_out, log_det
